# revision 6
# baseline (speedup 1.0000x reference)
"""VQ codebook encode+decode kernel for 8 Trainium2 NeuronCores.

Problem: x [4, 256, 4096] f32, e [8192, 256] f32.
  codes = argmin_k ||x[b,:,n] - e[k]||^2 ; out[b,:,n] = e[codes[b,n]]

Strategy (data-parallel over positions):
  - Shard N across the 8 cores (512 positions per batch per core, 2048 total).
  - Replicate the codebook. Host precomputes eT = e.T (matmul layout) and the
    per-code bias rows -0.5*(e2-256) split into fp16 hi+lo (exact to ~2^-21).
  - argmin_k d2 == argmax_k (x.e_k - e2_k/2): PE computes scores in PSUM
    (two 128-row d-half matmuls + one 2-row bias matmul per 512-wide k chunk),
    ACT copies PSUM->SBUF, DVE max/max_index gives the argmax index per row
    (first-match tie rule == jnp.argmin's lowest-index rule).
  - GPSIMD indirect DMA gathers the exact fp32 codebook rows, PE transposes
    [n,d] -> [d,n], and the result is DMAed to the output shard.
"""

import os

import numpy as np

import concourse.bass as bass
import concourse.bacc as bacc
import concourse.mybir as mybir
import concourse.tile as tile
from concourse.bass_utils import run_bass_kernel_spmd

B, D, N, K = 4, 256, 4096, 8192
NCORES = 8
NSH = N // NCORES          # 512 positions per batch per core
P = 128
KC = 512                   # k-chunk width (one PSUM bank of fp32)
PSG = 1024                 # psum group width (2 chunks per ACT drain)
NTILES = B * NSH // P      # 16 n-tiles per core

F32 = mybir.dt.float32
F32R = mybir.dt.float32r
F16 = mybir.dt.float16
U32 = mybir.dt.uint32

# Score-matmul precision mode: "f32" (exact, 4 cyc/row), "f32r" (1 cyc/row),
# "f16" (fp16 inputs, 1 cyc/row), "f16x2" (hi/lo fp16 emulation, 3 cyc/row).
MODE = os.environ.get("BASS_VQ_MODE", "f32")


def _mm_np_dtype(mode):
    return np.float16 if mode in ("f16", "f16x2") else np.float32


def _mm_bir_dtype(mode):
    return {"f32": F32, "f32r": F32R, "f16": F16, "f16x2": F16}[mode]


def build_bass(mode=MODE):
    nc = bacc.Bacc("TRN2", name="vq_codebook")
    mdt = _mm_bir_dtype(mode)
    nsplit = 2 if mode == "f16x2" else 1

    xs_d = nc.dram_tensor("xs", [nsplit, B, D, NSH], mdt, kind="ExternalInput")
    eT_d = nc.dram_tensor("eT", [nsplit, D, K], mdt, kind="ExternalInput")
    br_d = nc.dram_tensor("br", [2, K], F16, kind="ExternalInput")
    ones_d = nc.dram_tensor("ones2", [2, P], F16, kind="ExternalInput")
    ident_d = nc.dram_tensor("ident", [P, P], F32, kind="ExternalInput")
    e_d = nc.dram_tensor("e", [K, D], F32, kind="ExternalInput")
    out_d = nc.dram_tensor("out", [B, D, NSH], F32, kind="ExternalOutput")

    with tile.TileContext(nc) as tc:
        with (
            tc.tile_pool(name="const", bufs=1) as const,
            tc.tile_pool(name="scores_p", bufs=2) as scores_p,
            tc.tile_pool(name="small_p", bufs=2) as small_p,
            tc.tile_pool(name="ps_scores", bufs=3, space="PSUM") as ps_scores,
            tc.tile_pool(name="ps_tr", bufs=2, space="PSUM") as ps_tr,
        ):
            # --- constants / resident inputs ---
            # All loads go through the single SWDGE queue (gpsimd) so every
            # PE consumer needs at most ONE semaphore wait (walrus rejects
            # matmuls with >1 sync wait). Small consts first, then eT/xs.
            br_sb = const.tile([2, K], F16)
            nc.gpsimd.dma_start(out=br_sb[:], in_=br_d[:])
            ones_sb = const.tile([2, P], F16)
            nc.gpsimd.dma_start(out=ones_sb[:], in_=ones_d[:])
            ident = const.tile([P, P], F32)
            nc.gpsimd.dma_start(out=ident[:], in_=ident_d[:])
            eT_sb = const.tile([P, nsplit, 2, K], mdt)     # [p, split, d_half, k]
            for s in range(nsplit):
                nc.gpsimd.dma_start(
                    out=eT_sb[:, s],
                    in_=eT_d[s].rearrange("(h p) k -> p h k", h=2),
                )
            xs_sb = const.tile([P, nsplit, B, 2, NSH], mdt)  # [p, split, b, d_half, n]
            for s in range(nsplit):
                nc.gpsimd.dma_start(
                    out=xs_sb[:, s],
                    in_=xs_d[s].rearrange("b (h p) n -> p b h n", h=2),
                )

            # --- per n-tile pipeline ---
            for t in range(NTILES):
                b, j = divmod(t, NSH // P)
                n_sl = slice(j * P, (j + 1) * P)

                # (x-split, e-split) matmul term pairs; f16x2 drops lo*lo
                pairs = [(0, 0)] if nsplit == 1 else [(0, 0), (0, 1), (1, 0)]
                scores = scores_p.tile([P, K], F32)
                for g in range(K // PSG):
                    ps = ps_scores.tile([P, PSG], F32)
                    for c in range(PSG // KC):
                        k0 = g * PSG + c * KC
                        k_sl = slice(k0, k0 + KC)
                        o = ps[:, c * KC:(c + 1) * KC]
                        for i, (sx, se) in enumerate(pairs):
                            for h in range(2):
                                nc.tensor.matmul(
                                    o,
                                    lhsT=xs_sb[:, sx, b, h, n_sl],
                                    rhs=eT_sb[:, se, h, k_sl],
                                    start=(i == 0 and h == 0),
                                    stop=False,
                                )
                        nc.tensor.matmul(
                            o, lhsT=ones_sb[:, :], rhs=br_sb[:, k_sl],
                            start=False, stop=True,
                        )
                    nc.scalar.activation(
                        scores[:, g * PSG:(g + 1) * PSG], ps[:],
                        mybir.ActivationFunctionType.Copy,
                    )

                top8 = small_p.tile([P, 8], F32)
                idx8 = small_p.tile([P, 8], U32)
                nc.vector.max(out=top8[:], in_=scores[:])
                nc.vector.max_index(out=idx8[:], in_max=top8[:], in_values=scores[:])

                erows = small_p.tile([P, D], F32)
                nc.gpsimd.indirect_dma_start(
                    out=erows[:],
                    out_offset=None,
                    in_=e_d[:, :],
                    in_offset=bass.IndirectOffsetOnAxis(ap=idx8[:, :1], axis=0),
                )

                outT = small_p.tile([P, 2, P], F32)
                for h in range(2):
                    pst = ps_tr.tile([P, P], F32)
                    nc.tensor.transpose(
                        pst[:], erows[:, h * P:(h + 1) * P], ident[:]
                    )
                    nc.scalar.activation(
                        outT[:, h, :], pst[:], mybir.ActivationFunctionType.Copy
                    )
                for h in range(2):
                    nc.sync.dma_start(
                        out=out_d[b, h * P:(h + 1) * P, n_sl], in_=outT[:, h, :]
                    )
    nc.finalize()
    return nc


def _prep_inputs(x, e, mode=MODE):
    """Host-side prep: shard x over N, build eT / bias rows / ones."""
    mnp = _mm_np_dtype(mode)
    e2 = (e.astype(np.float64) ** 2).sum(axis=1)
    bias = (-0.5 * (e2 - 256.0)).astype(np.float32)
    b_hi = bias.astype(np.float16)
    b_lo = (bias - b_hi.astype(np.float32)).astype(np.float16)
    br = np.stack([b_hi, b_lo])                      # [2, K] f16
    ones2 = np.ones((2, P), dtype=np.float16)

    eT_f32 = np.ascontiguousarray(e.T)               # [D, K] f32
    if mode == "f16x2":
        eT_hi = eT_f32.astype(np.float16)
        eT_lo = (eT_f32 - eT_hi.astype(np.float32)).astype(np.float16)
        eT = np.stack([eT_hi, eT_lo])                # [2, D, K]
    else:
        eT = eT_f32.astype(mnp)[None]                # [1, D, K]

    in_maps = []
    for c in range(NCORES):
        xs_f32 = np.ascontiguousarray(x[:, :, c * NSH:(c + 1) * NSH])
        if mode == "f16x2":
            xs_hi = xs_f32.astype(np.float16)
            xs_lo = (xs_f32 - xs_hi.astype(np.float32)).astype(np.float16)
            xs = np.stack([xs_hi, xs_lo])            # [2, B, D, NSH]
        else:
            xs = xs_f32.astype(mnp)[None]            # [1, B, D, NSH]
        in_maps.append({
            "xs": np.ascontiguousarray(xs),
            "eT": eT,
            "br": br,
            "ones2": ones2,
            "ident": np.eye(P, dtype=np.float32),
            "e": e,
        })
    return in_maps


def run(x, e, mode=MODE, trace=False):
    nc = build_bass(mode)
    in_maps = _prep_inputs(x, e, mode)
    res = run_bass_kernel_spmd(
        nc, in_maps, core_ids=list(range(NCORES)), trace=trace
    )
    out = np.concatenate([r["out"] for r in res.results], axis=2)
    return out, res


def kernel(x, e):
    out, _ = run(np.asarray(x), np.asarray(e))
    return out


# revision 9
# speedup vs baseline: 1.8770x; 1.8770x over previous
"""VQ codebook encode+decode kernel for 8 Trainium2 NeuronCores.

Problem: x [4, 256, 4096] f32, e [8192, 256] f32.
  codes = argmin_k ||x[b,:,n] - e[k]||^2 ; out[b,:,n] = e[codes[b,n]]

Strategy (data-parallel over positions):
  - Shard N across the 8 cores (512 positions per batch per core, 2048 total).
  - Replicate the codebook. Host precomputes eT = e.T (matmul layout) and the
    per-code bias rows -0.5*(e2-256) split into fp16 hi+lo (exact to ~2^-21).
  - argmin_k d2 == argmax_k (x.e_k - e2_k/2): PE computes scores in PSUM
    (two 128-row d-half matmuls + one 2-row bias matmul per 512-wide k chunk),
    ACT copies PSUM->SBUF, DVE max/max_index gives the argmax index per row
    (first-match tie rule == jnp.argmin's lowest-index rule).
  - GPSIMD indirect DMA gathers the exact fp32 codebook rows, PE transposes
    [n,d] -> [d,n], and the result is DMAed to the output shard.
"""

import os

import numpy as np

import concourse.bass as bass
import concourse.bacc as bacc
import concourse.mybir as mybir
import concourse.tile as tile
from concourse.bass_utils import run_bass_kernel_spmd

B, D, N, K = 4, 256, 4096, 8192
NCORES = 8
NSH = N // NCORES          # 512 positions per batch per core
P = 128
KC = 512                   # k-chunk width (one PSUM bank of fp32)
PSG = 1024                 # psum group width (2 chunks per ACT drain)
NTILES = B * NSH // P      # 16 n-tiles per core

F32 = mybir.dt.float32
F32R = mybir.dt.float32r
F16 = mybir.dt.float16
U32 = mybir.dt.uint32

# Score-matmul precision mode: "f32" (exact, 4 cyc/row), "f32r" (1 cyc/row),
# "f16" (fp16 inputs, 1 cyc/row), "f16x2" (hi/lo fp16 emulation, 3 cyc/row).
MODE = os.environ.get("BASS_VQ_MODE", "f32r")


def _mm_np_dtype(mode):
    return np.float16 if mode in ("f16", "f16x2") else np.float32


def _mm_bir_dtype(mode):
    return {"f32": F32, "f32r": F32R, "f16": F16, "f16x2": F16}[mode]


def build_bass(mode=MODE):
    nc = bacc.Bacc("TRN2", name="vq_codebook")
    mdt = _mm_bir_dtype(mode)
    nsplit = 2 if mode == "f16x2" else 1

    xs_d = nc.dram_tensor("xs", [nsplit, B, D, NSH], mdt, kind="ExternalInput")
    eT_d = nc.dram_tensor("eT", [nsplit, D, K], mdt, kind="ExternalInput")
    br_d = nc.dram_tensor("br", [2, K], F16, kind="ExternalInput")
    ones_d = nc.dram_tensor("ones2", [2, P], F16, kind="ExternalInput")
    ident_d = nc.dram_tensor("ident", [P, P], F32, kind="ExternalInput")
    xt_d = nc.dram_tensor("xt", [B, NSH, D], F32, kind="ExternalInput")
    e_d = nc.dram_tensor("e", [K, D], F32, kind="ExternalInput")
    out_d = nc.dram_tensor("out", [B, D, NSH], F32, kind="ExternalOutput")

    rescue = mode in ("f32r", "f16")
    with tile.TileContext(nc) as tc:
        with (
            tc.tile_pool(name="const", bufs=1) as const,
            tc.tile_pool(name="scores_p", bufs=2) as scores_p,
            tc.tile_pool(name="small_p", bufs=2) as small_p,
            tc.tile_pool(name="resc_p", bufs=1) as resc_p,
            tc.tile_pool(name="ps_scores", bufs=3, space="PSUM") as ps_scores,
            tc.tile_pool(name="ps_tr", bufs=2, space="PSUM") as ps_tr,
        ):
            # --- constants / resident inputs ---
            # All loads go through the single SWDGE queue (gpsimd) so every
            # PE consumer needs at most ONE semaphore wait (walrus rejects
            # matmuls with >1 sync wait). Small consts first, then eT/xs.
            br_sb = const.tile([2, K], F16)
            nc.gpsimd.dma_start(out=br_sb[:], in_=br_d[:])
            ones_sb = const.tile([2, P], F16)
            nc.gpsimd.dma_start(out=ones_sb[:], in_=ones_d[:])
            ident = const.tile([P, P], F32)
            nc.gpsimd.dma_start(out=ident[:], in_=ident_d[:])
            eT_sb = const.tile([P, nsplit, 2, K], mdt)     # [p, split, d_half, k]
            for s in range(nsplit):
                nc.gpsimd.dma_start(
                    out=eT_sb[:, s],
                    in_=eT_d[s].rearrange("(h p) k -> p h k", h=2),
                )
            xs_sb = const.tile([P, nsplit, B, 2, NSH], mdt)  # [p, split, b, d_half, n]
            for s in range(nsplit):
                nc.gpsimd.dma_start(
                    out=xs_sb[:, s],
                    in_=xs_d[s].rearrange("b (h p) n -> p b h n", h=2),
                )

            # --- per n-tile pipeline ---
            for t in range(NTILES):
                b, j = divmod(t, NSH // P)
                n_sl = slice(j * P, (j + 1) * P)

                # (x-split, e-split) matmul term pairs; f16x2 drops lo*lo
                pairs = [(0, 0)] if nsplit == 1 else [(0, 0), (0, 1), (1, 0)]
                scores = scores_p.tile([P, K], F32)
                for g in range(K // PSG):
                    ps = ps_scores.tile([P, PSG], F32)
                    for c in range(PSG // KC):
                        k0 = g * PSG + c * KC
                        k_sl = slice(k0, k0 + KC)
                        o = ps[:, c * KC:(c + 1) * KC]
                        for i, (sx, se) in enumerate(pairs):
                            for h in range(2):
                                nc.tensor.matmul(
                                    o,
                                    lhsT=xs_sb[:, sx, b, h, n_sl],
                                    rhs=eT_sb[:, se, h, k_sl],
                                    start=(i == 0 and h == 0),
                                    stop=False,
                                )
                        nc.tensor.matmul(
                            o, lhsT=ones_sb[:, :], rhs=br_sb[:, k_sl],
                            start=False, stop=True,
                        )
                    nc.scalar.activation(
                        scores[:, g * PSG:(g + 1) * PSG], ps[:],
                        mybir.ActivationFunctionType.Copy,
                    )

                top8 = small_p.tile([P, 8], F32)
                idx8 = small_p.tile([P, 8], U32)
                if rescue:
                    # Pairwise-fold scores 8192 -> 4096 (2-port SBUF reads run
                    # at result rate), argmax over the folded array, then take
                    # BOTH elements of each of the top-4 folded pairs as the 8
                    # rescue candidates (the true max always survives folding).
                    mfold = scores_p.tile([P, K // 2], F32, name="mfold")
                    nc.vector.tensor_tensor(
                        out=mfold[:], in0=scores[:, 0:K:2], in1=scores[:, 1:K:2],
                        op=mybir.AluOpType.max)
                    nc.vector.max(out=top8[:], in_=mfold[:])
                    nc.vector.max_index(
                        out=idx8[:], in_max=top8[:], in_values=mfold[:])
                    fidxf = small_p.tile([P, 4], F32)
                    nc.vector.tensor_copy(fidxf[:], idx8[:, :4])
                    candf = small_p.tile([P, 8], F32)
                    nc.vector.tensor_scalar(
                        out=candf[:, 0:8:2], in0=fidxf[:], scalar1=2.0,
                        scalar2=None, op0=mybir.AluOpType.mult)
                    nc.vector.tensor_scalar(
                        out=candf[:, 1:8:2], in0=fidxf[:], scalar1=2.0,
                        scalar2=1.0, op0=mybir.AluOpType.mult,
                        op1=mybir.AluOpType.add)
                    cand8 = small_p.tile([P, 8], U32)
                    nc.vector.tensor_copy(cand8[:], candf[:])
                    idx8 = cand8
                else:
                    nc.vector.max(out=top8[:], in_=scores[:])
                    nc.vector.max_index(
                        out=idx8[:], in_max=top8[:], in_values=scores[:])

                if rescue:
                    # Exact top-8 rescue: the approx (f32r/f16) argmax can
                    # rank near-ties wrongly; rescore the top-8 candidates
                    # with exact fp32 d2 = sum((x - e_k)^2) and re-pick.
                    xt_t = small_p.tile([P, D], F32)
                    nc.sync.dma_start(out=xt_t[:], in_=xt_d[b, n_sl, :])
                    er8 = resc_p.tile([P, 8, D], F32)
                    for jj in range(8):
                        nc.gpsimd.indirect_dma_start(
                            out=er8[:, jj, :],
                            out_offset=None,
                            in_=e_d[:, :],
                            in_offset=bass.IndirectOffsetOnAxis(
                                ap=idx8[:, jj:jj + 1], axis=0),
                        )
                    # diff (in place, on Pool): er8 -= x
                    nc.gpsimd.tensor_tensor(
                        out=er8[:], in0=er8[:],
                        in1=xt_t[:, None, :].to_broadcast([P, 8, D]),
                        op=mybir.AluOpType.subtract,
                    )
                    # d2[:, j] = sum_d diff^2 (ACT square w/ accumulator)
                    d2 = small_p.tile([P, 8], F32)
                    for jj in range(8):
                        nc.scalar.activation(
                            er8[:, jj, :], er8[:, jj, :],
                            mybir.ActivationFunctionType.Square,
                            accum_out=d2[:, jj:jj + 1],
                        )
                    # exact winner among the 8; ties -> lowest original index
                    dmin = small_p.tile([P, 1], F32)
                    nc.vector.tensor_reduce(
                        dmin[:], d2[:], axis=mybir.AxisListType.X,
                        op=mybir.AluOpType.min)
                    mask = small_p.tile([P, 8], F32)
                    nc.vector.tensor_scalar(
                        out=mask[:], in0=d2[:], scalar1=dmin[:, :1],
                        scalar2=None, op0=mybir.AluOpType.is_le)
                    idxf = small_p.tile([P, 8], F32)
                    nc.vector.tensor_copy(idxf[:], idx8[:])
                    # penalty = (mask - 1) * (-1e9): 0 where mask, 1e9 else
                    nc.vector.tensor_scalar(
                        out=mask[:], in0=mask[:], scalar1=1.0, scalar2=-1e9,
                        op0=mybir.AluOpType.subtract,
                        op1=mybir.AluOpType.mult)
                    nc.vector.tensor_add(out=idxf[:], in0=idxf[:], in1=mask[:])
                    codef = small_p.tile([P, 1], F32)
                    nc.vector.tensor_reduce(
                        codef[:], idxf[:], axis=mybir.AxisListType.X,
                        op=mybir.AluOpType.min)
                    codes = small_p.tile([P, 1], U32)
                    nc.vector.tensor_copy(codes[:], codef[:])
                    gather_off = codes[:, :1]
                else:
                    gather_off = idx8[:, :1]

                erows = small_p.tile([P, D], F32)
                nc.gpsimd.indirect_dma_start(
                    out=erows[:],
                    out_offset=None,
                    in_=e_d[:, :],
                    in_offset=bass.IndirectOffsetOnAxis(ap=gather_off, axis=0),
                )

                outT = small_p.tile([P, 2, P], F32)
                for h in range(2):
                    pst = ps_tr.tile([P, P], F32)
                    nc.tensor.transpose(
                        pst[:], erows[:, h * P:(h + 1) * P], ident[:]
                    )
                    nc.scalar.activation(
                        outT[:, h, :], pst[:], mybir.ActivationFunctionType.Copy
                    )
                for h in range(2):
                    nc.sync.dma_start(
                        out=out_d[b, h * P:(h + 1) * P, n_sl], in_=outT[:, h, :]
                    )
    nc.finalize()
    return nc


def _prep_inputs(x, e, mode=MODE):
    """Host-side prep: shard x over N, build eT / bias rows / ones."""
    mnp = _mm_np_dtype(mode)
    e2 = (e.astype(np.float64) ** 2).sum(axis=1)
    bias = (-0.5 * (e2 - 256.0)).astype(np.float32)
    b_hi = bias.astype(np.float16)
    b_lo = (bias - b_hi.astype(np.float32)).astype(np.float16)
    br = np.stack([b_hi, b_lo])                      # [2, K] f16
    ones2 = np.ones((2, P), dtype=np.float16)

    eT_f32 = np.ascontiguousarray(e.T)               # [D, K] f32
    if mode == "f16x2":
        eT_hi = eT_f32.astype(np.float16)
        eT_lo = (eT_f32 - eT_hi.astype(np.float32)).astype(np.float16)
        eT = np.stack([eT_hi, eT_lo])                # [2, D, K]
    else:
        eT = eT_f32.astype(mnp)[None]                # [1, D, K]

    in_maps = []
    for c in range(NCORES):
        xs_f32 = np.ascontiguousarray(x[:, :, c * NSH:(c + 1) * NSH])
        if mode == "f16x2":
            xs_hi = xs_f32.astype(np.float16)
            xs_lo = (xs_f32 - xs_hi.astype(np.float32)).astype(np.float16)
            xs = np.stack([xs_hi, xs_lo])            # [2, B, D, NSH]
        else:
            xs = xs_f32.astype(mnp)[None]            # [1, B, D, NSH]
        xt = np.ascontiguousarray(xs_f32.transpose(0, 2, 1))  # [B, NSH, D]
        in_maps.append({
            "xs": np.ascontiguousarray(xs),
            "xt": xt,
            "eT": eT,
            "br": br,
            "ones2": ones2,
            "ident": np.eye(P, dtype=np.float32),
            "e": e,
        })
    return in_maps


def run(x, e, mode=MODE, trace=False):
    nc = build_bass(mode)
    in_maps = _prep_inputs(x, e, mode)
    res = run_bass_kernel_spmd(
        nc, in_maps, core_ids=list(range(NCORES)), trace=trace
    )
    out = np.concatenate([r["out"] for r in res.results], axis=2)
    return out, res


def kernel(x, e):
    out, _ = run(np.asarray(x), np.asarray(e))
    return out


# revision 11
# speedup vs baseline: 1.9813x; 1.0556x over previous
"""VQ codebook encode+decode kernel for 8 Trainium2 NeuronCores.

Problem: x [4, 256, 4096] f32, e [8192, 256] f32.
  codes = argmin_k ||x[b,:,n] - e[k]||^2 ; out[b,:,n] = e[codes[b,n]]

Strategy (data-parallel over positions):
  - Shard N across the 8 cores (512 positions per batch per core, 2048 total).
  - Replicate the codebook. Host precomputes eT = e.T (matmul layout) and the
    per-code bias rows -0.5*(e2-256) split into fp16 hi+lo (exact to ~2^-21).
  - argmin_k d2 == argmax_k (x.e_k - e2_k/2): PE computes scores in PSUM
    (two 128-row d-half matmuls + one 2-row bias matmul per 512-wide k chunk),
    ACT copies PSUM->SBUF, DVE max/max_index gives the argmax index per row
    (first-match tie rule == jnp.argmin's lowest-index rule).
  - GPSIMD indirect DMA gathers the exact fp32 codebook rows, PE transposes
    [n,d] -> [d,n], and the result is DMAed to the output shard.
"""

import os

import numpy as np

import concourse.bass as bass
import concourse.bacc as bacc
import concourse.mybir as mybir
import concourse.tile as tile
from concourse.bass_utils import run_bass_kernel_spmd

B, D, N, K = 4, 256, 4096, 8192
NCORES = 8
NSH = N // NCORES          # 512 positions per batch per core
P = 128
KC = 512                   # k-chunk width (one PSUM bank of fp32)
PSG = 1024                 # psum group width (2 chunks per ACT drain)
NTILES = B * NSH // P      # 16 n-tiles per core

F32 = mybir.dt.float32
F32R = mybir.dt.float32r
F16 = mybir.dt.float16
U32 = mybir.dt.uint32

# Score-matmul precision mode: "f32" (exact, 4 cyc/row), "f32r" (1 cyc/row),
# "f16" (fp16 inputs, 1 cyc/row), "f16x2" (hi/lo fp16 emulation, 3 cyc/row).
MODE = os.environ.get("BASS_VQ_MODE", "f32r")


def _mm_np_dtype(mode):
    return np.float16 if mode in ("f16", "f16x2") else np.float32


def _mm_bir_dtype(mode):
    return {"f32": F32, "f32r": F32R, "f16": F16, "f16x2": F16}[mode]


def build_bass(mode=MODE):
    nc = bacc.Bacc("TRN2", name="vq_codebook")
    mdt = _mm_bir_dtype(mode)
    nsplit = 2 if mode == "f16x2" else 1

    xs_d = nc.dram_tensor("xs", [nsplit, B, D, NSH], mdt, kind="ExternalInput")
    eT_d = nc.dram_tensor("eT", [nsplit, D, K], mdt, kind="ExternalInput")
    br_d = nc.dram_tensor("br", [2, K], F16, kind="ExternalInput")
    ones_d = nc.dram_tensor("ones2", [2, P], F16, kind="ExternalInput")
    ident_d = nc.dram_tensor("ident", [P, P], F32, kind="ExternalInput")
    xt_d = nc.dram_tensor("xt", [B, NSH, D], F32, kind="ExternalInput")
    e_d = nc.dram_tensor("e", [K, D], F32, kind="ExternalInput")
    out_d = nc.dram_tensor("out", [B, D, NSH], F32, kind="ExternalOutput")

    rescue = mode in ("f32r", "f16")
    with tile.TileContext(nc) as tc:
        with (
            tc.tile_pool(name="const", bufs=1) as const,
            tc.tile_pool(name="scores_p", bufs=2) as scores_p,
            tc.tile_pool(name="small_p", bufs=2) as small_p,
            tc.tile_pool(name="resc_p", bufs=1) as resc_p,
            tc.tile_pool(name="ps_scores", bufs=3, space="PSUM") as ps_scores,
            tc.tile_pool(name="ps_tr", bufs=2, space="PSUM") as ps_tr,
        ):
            # --- constants / resident inputs ---
            # All loads go through the single SWDGE queue (gpsimd) so every
            # PE consumer needs at most ONE semaphore wait (walrus rejects
            # matmuls with >1 sync wait). Small consts first, then eT/xs.
            br_sb = const.tile([2, K], F16)
            nc.gpsimd.dma_start(out=br_sb[:], in_=br_d[:])
            ones_sb = const.tile([2, P], F16)
            nc.gpsimd.dma_start(out=ones_sb[:], in_=ones_d[:])
            ident = const.tile([P, P], F32)
            nc.gpsimd.dma_start(out=ident[:], in_=ident_d[:])
            xs_sb = const.tile([P, nsplit, B, 2, NSH], mdt)  # [p, split, b, d_half, n]
            for s in range(nsplit):
                nc.gpsimd.dma_start(
                    out=xs_sb[:, s],
                    in_=xs_d[s].rearrange("b (h p) n -> p b h n", h=2),
                )
            eT_sb = const.tile([P, nsplit, 2, K], mdt)     # [p, split, d_half, k]
            for s in range(nsplit):
                eT_r = eT_d[s].rearrange("(h p) k -> p h k", h=2)
                for kq in range(8):
                    kq_sl = slice(kq * (K // 8), (kq + 1) * (K // 8))
                    for h in range(2):
                        nc.gpsimd.dma_start(
                            out=eT_sb[:, s, h, kq_sl], in_=eT_r[:, h, kq_sl]
                        )

            # --- per n-tile pipeline ---
            for t in range(NTILES):
                b, j = divmod(t, NSH // P)
                n_sl = slice(j * P, (j + 1) * P)

                # (x-split, e-split) matmul term pairs; f16x2 drops lo*lo
                pairs = [(0, 0)] if nsplit == 1 else [(0, 0), (0, 1), (1, 0)]
                scores = scores_p.tile([P, K], F16 if rescue else F32)
                for g in range(K // PSG):
                    ps = ps_scores.tile([P, PSG], F32)
                    for c in range(PSG // KC):
                        k0 = g * PSG + c * KC
                        k_sl = slice(k0, k0 + KC)
                        o = ps[:, c * KC:(c + 1) * KC]
                        for i, (sx, se) in enumerate(pairs):
                            for h in range(2):
                                nc.tensor.matmul(
                                    o,
                                    lhsT=xs_sb[:, sx, b, h, n_sl],
                                    rhs=eT_sb[:, se, h, k_sl],
                                    start=(i == 0 and h == 0),
                                    stop=False,
                                )
                        nc.tensor.matmul(
                            o, lhsT=ones_sb[:, :], rhs=br_sb[:, k_sl],
                            start=False, stop=True,
                        )
                    nc.scalar.activation(
                        scores[:, g * PSG:(g + 1) * PSG], ps[:],
                        mybir.ActivationFunctionType.Copy,
                    )

                top8 = small_p.tile([P, 8], F16 if rescue else F32)
                idx8 = small_p.tile([P, 8], U32)
                if rescue:
                    # Pairwise-fold scores 8192 -> 4096 (2-port SBUF reads run
                    # at result rate), argmax over the folded array, then take
                    # BOTH elements of each of the top-4 folded pairs as the 8
                    # rescue candidates (the true max always survives folding).
                    mfold = scores_p.tile([P, K // 2], F16, name="mfold")
                    nc.vector.tensor_tensor(
                        out=mfold[:], in0=scores[:, :K // 2],
                        in1=scores[:, K // 2:], op=mybir.AluOpType.max)
                    nc.vector.max(out=top8[:], in_=mfold[:])
                    nc.vector.max_index(
                        out=idx8[:], in_max=top8[:], in_values=mfold[:])
                    # candidates: {j, j + K/2} for the top-4 folded slots
                    fidxf = small_p.tile([P, 4], F32)
                    nc.vector.tensor_copy(fidxf[:], idx8[:, :4])
                    candf = small_p.tile([P, 8], F32)
                    nc.vector.tensor_copy(candf[:, 0:4], fidxf[:])
                    nc.vector.tensor_scalar(
                        out=candf[:, 4:8], in0=fidxf[:], scalar1=float(K // 2),
                        scalar2=None, op0=mybir.AluOpType.add)
                    cand8 = small_p.tile([P, 8], U32)
                    nc.vector.tensor_copy(cand8[:], candf[:])
                    idx8 = cand8
                else:
                    nc.vector.max(out=top8[:], in_=scores[:])
                    nc.vector.max_index(
                        out=idx8[:], in_max=top8[:], in_values=scores[:])

                if rescue:
                    # Exact top-8 rescue: the approx (f32r/f16) argmax can
                    # rank near-ties wrongly; rescore the top-8 candidates
                    # with exact fp32 d2 = sum((x - e_k)^2) and re-pick.
                    xt_t = small_p.tile([P, D], F32)
                    nc.sync.dma_start(out=xt_t[:], in_=xt_d[b, n_sl, :])
                    er8 = resc_p.tile([P, 8, D], F32)
                    for jj in range(8):
                        nc.gpsimd.indirect_dma_start(
                            out=er8[:, jj, :],
                            out_offset=None,
                            in_=e_d[:, :],
                            in_offset=bass.IndirectOffsetOnAxis(
                                ap=idx8[:, jj:jj + 1], axis=0),
                        )
                    # diff (in place, on Pool): er8 -= x
                    nc.gpsimd.tensor_tensor(
                        out=er8[:], in0=er8[:],
                        in1=xt_t[:, None, :].to_broadcast([P, 8, D]),
                        op=mybir.AluOpType.subtract,
                    )
                    # d2[:, j] = sum_d diff^2 (ACT square w/ accumulator)
                    d2 = small_p.tile([P, 8], F32)
                    for jj in range(8):
                        nc.scalar.activation(
                            er8[:, jj, :], er8[:, jj, :],
                            mybir.ActivationFunctionType.Square,
                            accum_out=d2[:, jj:jj + 1],
                        )
                    # exact winner among the 8; ties -> lowest original index
                    dmin = small_p.tile([P, 1], F32)
                    nc.vector.tensor_reduce(
                        dmin[:], d2[:], axis=mybir.AxisListType.X,
                        op=mybir.AluOpType.min)
                    mask = small_p.tile([P, 8], F32)
                    nc.vector.tensor_scalar(
                        out=mask[:], in0=d2[:], scalar1=dmin[:, :1],
                        scalar2=None, op0=mybir.AluOpType.is_le)
                    idxf = small_p.tile([P, 8], F32)
                    nc.vector.tensor_copy(idxf[:], idx8[:])
                    # penalty = (mask - 1) * (-1e9): 0 where mask, 1e9 else
                    nc.vector.tensor_scalar(
                        out=mask[:], in0=mask[:], scalar1=1.0, scalar2=-1e9,
                        op0=mybir.AluOpType.subtract,
                        op1=mybir.AluOpType.mult)
                    nc.vector.tensor_add(out=idxf[:], in0=idxf[:], in1=mask[:])
                    codef = small_p.tile([P, 1], F32)
                    nc.vector.tensor_reduce(
                        codef[:], idxf[:], axis=mybir.AxisListType.X,
                        op=mybir.AluOpType.min)
                    codes = small_p.tile([P, 1], U32)
                    nc.vector.tensor_copy(codes[:], codef[:])
                    gather_off = codes[:, :1]
                else:
                    gather_off = idx8[:, :1]

                erows = small_p.tile([P, D], F32)
                nc.gpsimd.indirect_dma_start(
                    out=erows[:],
                    out_offset=None,
                    in_=e_d[:, :],
                    in_offset=bass.IndirectOffsetOnAxis(ap=gather_off, axis=0),
                )

                outT = small_p.tile([P, 2, P], F32)
                for h in range(2):
                    pst = ps_tr.tile([P, P], F32)
                    nc.tensor.transpose(
                        pst[:], erows[:, h * P:(h + 1) * P], ident[:]
                    )
                    nc.scalar.activation(
                        outT[:, h, :], pst[:], mybir.ActivationFunctionType.Copy
                    )
                for h in range(2):
                    nc.sync.dma_start(
                        out=out_d[b, h * P:(h + 1) * P, n_sl], in_=outT[:, h, :]
                    )
    nc.finalize()
    return nc


def _prep_inputs(x, e, mode=MODE):
    """Host-side prep: shard x over N, build eT / bias rows / ones."""
    mnp = _mm_np_dtype(mode)
    e2 = (e.astype(np.float64) ** 2).sum(axis=1)
    bias = (-0.5 * (e2 - 256.0)).astype(np.float32)
    b_hi = bias.astype(np.float16)
    b_lo = (bias - b_hi.astype(np.float32)).astype(np.float16)
    br = np.stack([b_hi, b_lo])                      # [2, K] f16
    ones2 = np.ones((2, P), dtype=np.float16)

    eT_f32 = np.ascontiguousarray(e.T)               # [D, K] f32
    if mode == "f16x2":
        eT_hi = eT_f32.astype(np.float16)
        eT_lo = (eT_f32 - eT_hi.astype(np.float32)).astype(np.float16)
        eT = np.stack([eT_hi, eT_lo])                # [2, D, K]
    else:
        eT = eT_f32.astype(mnp)[None]                # [1, D, K]

    in_maps = []
    for c in range(NCORES):
        xs_f32 = np.ascontiguousarray(x[:, :, c * NSH:(c + 1) * NSH])
        if mode == "f16x2":
            xs_hi = xs_f32.astype(np.float16)
            xs_lo = (xs_f32 - xs_hi.astype(np.float32)).astype(np.float16)
            xs = np.stack([xs_hi, xs_lo])            # [2, B, D, NSH]
        else:
            xs = xs_f32.astype(mnp)[None]            # [1, B, D, NSH]
        xt = np.ascontiguousarray(xs_f32.transpose(0, 2, 1))  # [B, NSH, D]
        in_maps.append({
            "xs": np.ascontiguousarray(xs),
            "xt": xt,
            "eT": eT,
            "br": br,
            "ones2": ones2,
            "ident": np.eye(P, dtype=np.float32),
            "e": e,
        })
    return in_maps


def run(x, e, mode=MODE, trace=False):
    nc = build_bass(mode)
    in_maps = _prep_inputs(x, e, mode)
    res = run_bass_kernel_spmd(
        nc, in_maps, core_ids=list(range(NCORES)), trace=trace
    )
    out = np.concatenate([r["out"] for r in res.results], axis=2)
    return out, res


def kernel(x, e):
    out, _ = run(np.asarray(x), np.asarray(e))
    return out


# revision 12
# speedup vs baseline: 2.4703x; 1.2468x over previous
"""VQ codebook encode+decode kernel for 8 Trainium2 NeuronCores.

Problem: x [4, 256, 4096] f32, e [8192, 256] f32.
  codes = argmin_k ||x[b,:,n] - e[k]||^2 ; out[b,:,n] = e[codes[b,n]]

Strategy (data-parallel over positions):
  - Shard N across the 8 cores (512 positions per batch per core, 2048 total).
  - Replicate the codebook. Host precomputes eT = e.T (matmul layout) and the
    per-code bias rows -0.5*(e2-256) split into fp16 hi+lo (exact to ~2^-21).
  - argmin_k d2 == argmax_k (x.e_k - e2_k/2): PE computes scores in PSUM
    (two 128-row d-half matmuls + one 2-row bias matmul per 512-wide k chunk),
    ACT copies PSUM->SBUF, DVE max/max_index gives the argmax index per row
    (first-match tie rule == jnp.argmin's lowest-index rule).
  - GPSIMD indirect DMA gathers the exact fp32 codebook rows, PE transposes
    [n,d] -> [d,n], and the result is DMAed to the output shard.
"""

import os

import numpy as np

import concourse.bass as bass
import concourse.bacc as bacc
import concourse.mybir as mybir
import concourse.tile as tile
from concourse.bass_utils import run_bass_kernel_spmd

B, D, N, K = 4, 256, 4096, 8192
NCORES = 8
NSH = N // NCORES          # 512 positions per batch per core
P = 128
KC = 512                   # k-chunk width (one PSUM bank of fp32)
PSG = 1024                 # psum group width (2 chunks per ACT drain)
NTILES = B * NSH // P      # 16 n-tiles per core

F32 = mybir.dt.float32
F32R = mybir.dt.float32r
F16 = mybir.dt.float16
U32 = mybir.dt.uint32

# Score-matmul precision mode: "f32" (exact, 4 cyc/row), "f32r" (1 cyc/row),
# "f16" (fp16 inputs, 1 cyc/row), "f16x2" (hi/lo fp16 emulation, 3 cyc/row).
MODE = os.environ.get("BASS_VQ_MODE", "f32r")


def _mm_np_dtype(mode):
    return np.float16 if mode in ("f16", "f16x2") else np.float32


def _mm_bir_dtype(mode):
    return {"f32": F32, "f32r": F32R, "f16": F16, "f16x2": F16}[mode]


def build_bass(mode=MODE):
    nc = bacc.Bacc("TRN2", name="vq_codebook")
    mdt = _mm_bir_dtype(mode)
    nsplit = 2 if mode == "f16x2" else 1

    xs_d = nc.dram_tensor("xs", [nsplit, B, D, NSH], mdt, kind="ExternalInput")
    eT_d = nc.dram_tensor("eT", [nsplit, D, K], mdt, kind="ExternalInput")
    br_d = nc.dram_tensor("br", [2, K], F16, kind="ExternalInput")
    ones_d = nc.dram_tensor("ones2", [2, P], F16, kind="ExternalInput")
    ident_d = nc.dram_tensor("ident", [P, P], F32, kind="ExternalInput")
    xt_d = nc.dram_tensor("xt", [B, NSH, D], F32, kind="ExternalInput")
    e_d = nc.dram_tensor("e", [K, D], F32, kind="ExternalInput")
    out_d = nc.dram_tensor("out", [B, D, NSH], F32, kind="ExternalOutput")

    rescue = mode in ("f32r", "f16")
    with tile.TileContext(nc) as tc:
        with (
            tc.tile_pool(name="const", bufs=1) as const,
            tc.tile_pool(name="scores_p", bufs=2) as scores_p,
            tc.tile_pool(name="small_p", bufs=2) as small_p,
            tc.tile_pool(name="resc_p", bufs=1) as resc_p,
            tc.tile_pool(name="ps_scores", bufs=3, space="PSUM") as ps_scores,
            tc.tile_pool(name="ps_tr", bufs=2, space="PSUM") as ps_tr,
        ):
            # --- constants / resident inputs ---
            # All loads go through the single SWDGE queue (gpsimd) so every
            # PE consumer needs at most ONE semaphore wait (walrus rejects
            # matmuls with >1 sync wait). Small consts first, then eT/xs.
            br_sb = const.tile([2, K], F16)
            nc.gpsimd.dma_start(out=br_sb[:], in_=br_d[:])
            ones_sb = const.tile([2, P], F16)
            nc.gpsimd.dma_start(out=ones_sb[:], in_=ones_d[:])
            ident = const.tile([P, P], F32)
            nc.gpsimd.dma_start(out=ident[:], in_=ident_d[:])
            xs_sb = const.tile([P, nsplit, B, 2, NSH], mdt)  # [p, split, b, d_half, n]
            for s in range(nsplit):
                nc.gpsimd.dma_start(
                    out=xs_sb[:, s],
                    in_=xs_d[s].rearrange("b (h p) n -> p b h n", h=2),
                )
            eT_sb = const.tile([P, nsplit, 2, K], mdt)     # [p, split, d_half, k]
            for s in range(nsplit):
                eT_r = eT_d[s].rearrange("(h p) k -> p h k", h=2)
                for kq in range(8):
                    kq_sl = slice(kq * (K // 8), (kq + 1) * (K // 8))
                    for h in range(2):
                        nc.gpsimd.dma_start(
                            out=eT_sb[:, s, h, kq_sl], in_=eT_r[:, h, kq_sl]
                        )

            # --- per n-tile pipeline ---
            for t in range(NTILES):
                b, j = divmod(t, NSH // P)
                n_sl = slice(j * P, (j + 1) * P)

                # (x-split, e-split) matmul term pairs; f16x2 drops lo*lo
                pairs = [(0, 0)] if nsplit == 1 else [(0, 0), (0, 1), (1, 0)]
                scores = scores_p.tile([P, K], F16 if rescue else F32)
                for g in range(K // PSG):
                    ps = ps_scores.tile([P, PSG], F32)
                    for c in range(PSG // KC):
                        k0 = g * PSG + c * KC
                        k_sl = slice(k0, k0 + KC)
                        o = ps[:, c * KC:(c + 1) * KC]
                        for i, (sx, se) in enumerate(pairs):
                            for h in range(2):
                                nc.tensor.matmul(
                                    o,
                                    lhsT=xs_sb[:, sx, b, h, n_sl],
                                    rhs=eT_sb[:, se, h, k_sl],
                                    start=(i == 0 and h == 0),
                                    stop=False,
                                )
                        nc.tensor.matmul(
                            o, lhsT=ones_sb[:, :], rhs=br_sb[:, k_sl],
                            start=False, stop=True,
                        )
                    nc.scalar.activation(
                        scores[:, g * PSG:(g + 1) * PSG], ps[:],
                        mybir.ActivationFunctionType.Copy,
                    )

                top8 = small_p.tile([P, 8], F16 if rescue else F32)
                idx8 = small_p.tile([P, 8], U32)
                if rescue:
                    # Pairwise-fold scores 8192 -> 4096 (2-port SBUF reads run
                    # at result rate), argmax over the folded array, then take
                    # BOTH elements of each of the top-4 folded pairs as the 8
                    # rescue candidates (the true max always survives folding).
                    mfold = scores_p.tile([P, K // 2], F16, name="mfold")
                    nc.vector.tensor_tensor(
                        out=mfold[:], in0=scores[:, :K // 2],
                        in1=scores[:, K // 2:], op=mybir.AluOpType.max)
                    nc.vector.max(out=top8[:], in_=mfold[:])
                    nc.vector.max_index(
                        out=idx8[:], in_max=top8[:], in_values=mfold[:])
                    # candidates: {j, j + K/2} for the top-4 folded slots
                    fidxf = small_p.tile([P, 4], F32)
                    nc.vector.tensor_copy(fidxf[:], idx8[:, :4])
                    candf = small_p.tile([P, 8], F32)
                    nc.vector.tensor_copy(candf[:, 0:4], fidxf[:])
                    nc.vector.tensor_scalar(
                        out=candf[:, 4:8], in0=fidxf[:], scalar1=float(K // 2),
                        scalar2=None, op0=mybir.AluOpType.add)
                    cand8 = small_p.tile([P, 8], U32)
                    nc.vector.tensor_copy(cand8[:], candf[:])
                    idx8 = cand8
                else:
                    nc.vector.max(out=top8[:], in_=scores[:])
                    nc.vector.max_index(
                        out=idx8[:], in_max=top8[:], in_values=scores[:])

                if rescue:
                    # Exact top-8 rescue: the approx (f32r/f16) argmax can
                    # rank near-ties wrongly; rescore the top-8 candidates
                    # with exact fp32 d2 = sum((x - e_k)^2) and re-pick.
                    xt_t = small_p.tile([P, D], F32)
                    nc.sync.dma_start(out=xt_t[:], in_=xt_d[b, n_sl, :])
                    er8 = resc_p.tile([P, 8, D], F32)
                    for jj in range(8):
                        nc.gpsimd.indirect_dma_start(
                            out=er8[:, jj, :],
                            out_offset=None,
                            in_=e_d[:, :],
                            in_offset=bass.IndirectOffsetOnAxis(
                                ap=idx8[:, jj:jj + 1], axis=0),
                        )
                    # diff (in place, on Pool): er8 -= x
                    nc.gpsimd.tensor_tensor(
                        out=er8[:], in0=er8[:],
                        in1=xt_t[:, None, :].to_broadcast([P, 8, D]),
                        op=mybir.AluOpType.subtract,
                    )
                    # d2[:, j] = sum_d diff^2 — candidates 0-4 on ACT
                    # (Square w/ accumulator), 5-7 squared on Pool with the
                    # row-sums on DVE, to balance engine load.
                    d2 = small_p.tile([P, 8], F32)
                    for jj in range(5):
                        nc.scalar.activation(
                            er8[:, jj, :], er8[:, jj, :],
                            mybir.ActivationFunctionType.Square,
                            accum_out=d2[:, jj:jj + 1],
                        )
                    nc.gpsimd.tensor_tensor(
                        out=er8[:, 5:8, :], in0=er8[:, 5:8, :],
                        in1=er8[:, 5:8, :], op=mybir.AluOpType.mult)
                    nc.vector.tensor_reduce(
                        d2[:, 5:8], er8[:, 5:8, :], axis=mybir.AxisListType.X,
                        op=mybir.AluOpType.add)
                    # exact winner among the 8; ties -> lowest original index
                    dmin = small_p.tile([P, 1], F32)
                    nc.vector.tensor_reduce(
                        dmin[:], d2[:], axis=mybir.AxisListType.X,
                        op=mybir.AluOpType.min)
                    mask = small_p.tile([P, 8], F32)
                    nc.vector.tensor_scalar(
                        out=mask[:], in0=d2[:], scalar1=dmin[:, :1],
                        scalar2=None, op0=mybir.AluOpType.is_le)
                    idxf = small_p.tile([P, 8], F32)
                    nc.vector.tensor_copy(idxf[:], idx8[:])
                    # penalty = (mask - 1) * (-1e9): 0 where mask, 1e9 else
                    nc.vector.tensor_scalar(
                        out=mask[:], in0=mask[:], scalar1=1.0, scalar2=-1e9,
                        op0=mybir.AluOpType.subtract,
                        op1=mybir.AluOpType.mult)
                    nc.vector.tensor_add(out=idxf[:], in0=idxf[:], in1=mask[:])
                    codef = small_p.tile([P, 1], F32)
                    nc.vector.tensor_reduce(
                        codef[:], idxf[:], axis=mybir.AxisListType.X,
                        op=mybir.AluOpType.min)
                    codes = small_p.tile([P, 1], U32)
                    nc.vector.tensor_copy(codes[:], codef[:])
                    gather_off = codes[:, :1]
                else:
                    gather_off = idx8[:, :1]

                erows = small_p.tile([P, D], F32)
                nc.gpsimd.indirect_dma_start(
                    out=erows[:],
                    out_offset=None,
                    in_=e_d[:, :],
                    in_offset=bass.IndirectOffsetOnAxis(ap=gather_off, axis=0),
                )

                outT = small_p.tile([P, 2, P], F32)
                for h in range(2):
                    pst = ps_tr.tile([P, P], F32)
                    nc.tensor.transpose(
                        pst[:], erows[:, h * P:(h + 1) * P], ident[:]
                    )
                    nc.scalar.activation(
                        outT[:, h, :], pst[:], mybir.ActivationFunctionType.Copy
                    )
                for h in range(2):
                    nc.sync.dma_start(
                        out=out_d[b, h * P:(h + 1) * P, n_sl], in_=outT[:, h, :]
                    )
    nc.finalize()
    return nc


def _prep_inputs(x, e, mode=MODE):
    """Host-side prep: shard x over N, build eT / bias rows / ones."""
    mnp = _mm_np_dtype(mode)
    e2 = (e.astype(np.float64) ** 2).sum(axis=1)
    bias = (-0.5 * (e2 - 256.0)).astype(np.float32)
    b_hi = bias.astype(np.float16)
    b_lo = (bias - b_hi.astype(np.float32)).astype(np.float16)
    br = np.stack([b_hi, b_lo])                      # [2, K] f16
    ones2 = np.ones((2, P), dtype=np.float16)

    eT_f32 = np.ascontiguousarray(e.T)               # [D, K] f32
    if mode == "f16x2":
        eT_hi = eT_f32.astype(np.float16)
        eT_lo = (eT_f32 - eT_hi.astype(np.float32)).astype(np.float16)
        eT = np.stack([eT_hi, eT_lo])                # [2, D, K]
    else:
        eT = eT_f32.astype(mnp)[None]                # [1, D, K]

    in_maps = []
    for c in range(NCORES):
        xs_f32 = np.ascontiguousarray(x[:, :, c * NSH:(c + 1) * NSH])
        if mode == "f16x2":
            xs_hi = xs_f32.astype(np.float16)
            xs_lo = (xs_f32 - xs_hi.astype(np.float32)).astype(np.float16)
            xs = np.stack([xs_hi, xs_lo])            # [2, B, D, NSH]
        else:
            xs = xs_f32.astype(mnp)[None]            # [1, B, D, NSH]
        xt = np.ascontiguousarray(xs_f32.transpose(0, 2, 1))  # [B, NSH, D]
        in_maps.append({
            "xs": np.ascontiguousarray(xs),
            "xt": xt,
            "eT": eT,
            "br": br,
            "ones2": ones2,
            "ident": np.eye(P, dtype=np.float32),
            "e": e,
        })
    return in_maps


def run(x, e, mode=MODE, trace=False):
    nc = build_bass(mode)
    in_maps = _prep_inputs(x, e, mode)
    res = run_bass_kernel_spmd(
        nc, in_maps, core_ids=list(range(NCORES)), trace=trace
    )
    out = np.concatenate([r["out"] for r in res.results], axis=2)
    return out, res


def kernel(x, e):
    out, _ = run(np.asarray(x), np.asarray(e))
    return out
